# revision 2
# baseline (speedup 1.0000x reference)
"""Causal flash attention for trn2: B=4,H=16,S=4096,D=64 fp32.

Sharding: 64 (b,h) heads -> 8 per NeuronCore, no cross-core comm.
Host prep (not counted in HW time): Q/K transposed to [d,s] bf16 (Q
pre-scaled by 1/sqrt(D)), V pre-laid-out [128, nkt, D+1] bf16 with an
appended ones-column so the PV matmul also produces the softmax
normalizer.

On-chip per head, per 512-query block j (PSUM: 3x double-buffered
2-bank score sets + 2 single-bank output accumulators = 8 banks):
  key tiles t<=4j+3 processed in groups of 2:
    ST[k=128, 1024] = two QK matmuls (contraction d=64, bf16, N=512)
    exp in ONE instruction per group, alternating engines:
      ACT: exp activation (table)         -> pt bf16
      DVE: Schraudolph bit-trick exp (x*A+B -> int16, bits = bf16)
    diagonal tiles: pt[:, dg*128:+128] *= upper-tri mask (DVE)
    PV: per 128-query sub s: O[q=128, s*65..+65] += PT_s^T @ [V_t|1]
        (single PSUM bank holds all 4 sub-accumulators; one start=True
        clears has_written once, later first-touches overwrite)
  epilogue: strided reciprocal of the 4 normalizer cols, 4 scaled
  copies, one DMA of [128, 4, 64] -> out rows.
"""

import math
from contextlib import ExitStack

import numpy as np
import ml_dtypes

B, H, S, D = 4, 16, 4096, 64
NCORES = 8
HPC = (B * H) // NCORES  # heads per core
QB = 512                 # query block
KT = 128                 # key tile (PE partition dim)
NKT = S // KT            # 32 key tiles per head
GT = 2                   # key tiles per exp group
DV = D + 1               # value cols + normalizer ones-column

# Schraudolph exp constants for bf16 bit pattern (7-bit mantissa):
# bits = round(x * 128/ln2 + (127*128 - 366393/65536))
SCH_A = 128.0 / math.log(2.0)
SCH_B = 127 * 128 - 366393.0 / 65536.0

# exp engine schedule: cycle over groups; ~3:2 ACT:DVE
EXP_PATTERN = ("act", "dve", "act", "dve", "act")

_cache = {}


def _build(causal: bool, hpc: int = HPC, s_len: int = S):
    import concourse.tile as tile
    from concourse import bacc, mybir

    f32 = mybir.dt.float32
    bf16 = mybir.dt.bfloat16
    i16 = mybir.dt.int16
    EXP = mybir.ActivationFunctionType.Exp
    MULT = mybir.AluOpType.mult
    ADD = mybir.AluOpType.add
    nkt_total = s_len // KT
    nqb = s_len // QB

    nc = bacc.Bacc("TRN2", target_bir_lowering=False)
    qt_d = nc.dram_tensor("qt", [hpc, D, s_len], bf16, kind="ExternalInput")
    kt_d = nc.dram_tensor("kt", [hpc, D, s_len], bf16, kind="ExternalInput")
    v_d = nc.dram_tensor("v", [hpc, KT, nkt_total, DV], bf16, kind="ExternalInput")
    tri_d = nc.dram_tensor("tri", [KT, KT], bf16, kind="ExternalInput")
    o_d = nc.dram_tensor("o", [hpc, s_len, D], f32, kind="ExternalOutput")

    with ExitStack() as ctx:
        tc = ctx.enter_context(tile.TileContext(nc))
        qk_pool = ctx.enter_context(tc.tile_pool(name="qk", bufs=2))
        v_pool = ctx.enter_context(tc.tile_pool(name="v", bufs=2))
        p_pool = ctx.enter_context(tc.tile_pool(name="p", bufs=3))
        st_pool = ctx.enter_context(tc.tile_pool(name="st", bufs=3, space="PSUM"))
        o_pool = ctx.enter_context(tc.tile_pool(name="oacc", bufs=2, space="PSUM"))
        out_pool = ctx.enter_context(tc.tile_pool(name="out", bufs=4))
        const_pool = ctx.enter_context(tc.tile_pool(name="const", bufs=1))

        tri_t = const_pool.tile([KT, KT], bf16)
        nc.sync.dma_start(out=tri_t, in_=tri_d[:])

        gcounter = 0
        for h in range(hpc):
            qt_t = qk_pool.tile([D, s_len], bf16, tag="qt")
            nc.sync.dma_start(out=qt_t, in_=qt_d[h])
            kt_t = qk_pool.tile([D, s_len], bf16, tag="kt")
            nc.sync.dma_start(out=kt_t, in_=kt_d[h])
            v_t = v_pool.tile([KT, nkt_total, DV], bf16, tag="v")
            nc.sync.dma_start(out=v_t, in_=v_d[h])

            for j in range(nqb):
                o_ps = o_pool.tile([KT, 4 * DV], f32, tag="oacc", name=f"o_{h}_{j}")
                nkt = 4 * (j + 1) if causal else nkt_total
                groups = [
                    list(range(g0, min(g0 + GT, nkt))) for g0 in range(0, nkt, GT)
                ]
                # stage 1: QK matmuls + exp per group; software-pipelined by
                # emission order (tile schedules by deps, PE issues in order)
                pts = []
                for ts in groups:
                    w = len(ts) * QB
                    st = st_pool.tile([KT, GT * QB], f32, tag="st")
                    for i, t in enumerate(ts):
                        nc.tensor.matmul(
                            st[:, i * QB:(i + 1) * QB],
                            kt_t[:, t * KT:(t + 1) * KT],
                            qt_t[:, j * QB:(j + 1) * QB],
                            start=True, stop=True,
                        )
                    pt = p_pool.tile([KT, GT * QB], bf16, tag="pt")
                    if EXP_PATTERN[gcounter % len(EXP_PATTERN)] == "dve":
                        nc.vector.tensor_scalar(
                            pt.bitcast(i16)[:, :w], st[:, :w], SCH_A, SCH_B,
                            MULT, ADD,
                        )
                    else:
                        nc.scalar.activation(pt[:, :w], st[:, :w], EXP)
                    gcounter += 1
                    for i, t in enumerate(ts):
                        dg = t - 4 * j if causal else -1
                        if dg >= 0:
                            c0 = i * QB + dg * KT
                            nc.vector.tensor_mul(
                                pt[:, c0:c0 + KT], pt[:, c0:c0 + KT], tri_t
                            )
                    pts.append((ts, pt))
                    # PV for the PREVIOUS group (1-group pipeline lag keeps
                    # PE fed with QK matmuls while exp of this group runs)
                    if len(pts) >= 2:
                        _emit_pv(nc, causal, j, pts.pop(0), o_ps, v_t)
                for rem in pts:
                    _emit_pv(nc, causal, j, rem, o_ps, v_t)

                # epilogue: normalize + store
                recip = out_pool.tile([KT, 4], f32, tag="recip")
                nc.vector.reciprocal(recip, o_ps[:, D::DV])
                out_t = out_pool.tile([KT, 4, D], f32, tag="out")
                for s in range(4):
                    nc.vector.tensor_scalar_mul(
                        out_t[:, s], o_ps[:, s * DV:s * DV + D], recip[:, s:s + 1]
                    )
                nc.sync.dma_start(
                    out=o_d[h, j * QB:(j + 1) * QB, :].rearrange(
                        "(s p) d -> p s d", s=4
                    ),
                    in_=out_t,
                )
    nc.compile()
    return nc


def _emit_pv(nc, causal, j, group, o_ps, v_t):
    ts, pt = group
    for i, t in enumerate(ts):
        dg = t - 4 * j if causal else -1
        for s in range(4):
            if dg > s:
                continue
            nc.tensor.matmul(
                o_ps[:, s * DV:(s + 1) * DV],
                pt[:, i * QB + s * KT:i * QB + (s + 1) * KT],
                v_t[:, t, :],
                start=(t == 0 and s == 0),
                stop=(t == (4 * j + s if causal else NKT - 1)),
            )


last_results = None  # BassKernelResults of the most recent run (for test.py)


def _make_in_maps(query, key, value):
    bf = ml_dtypes.bfloat16
    q4 = np.asarray(query, dtype=np.float32).reshape(B * H, S, D)
    k4 = np.asarray(key, dtype=np.float32).reshape(B * H, S, D)
    v4 = np.asarray(value, dtype=np.float32).reshape(B * H, S, D)
    tri = np.triu(np.ones((KT, KT), dtype=np.float32)).astype(bf)

    in_maps = []
    for c in range(NCORES):
        sl = slice(c * HPC, (c + 1) * HPC)
        qt = np.ascontiguousarray(
            (q4[sl] / math.sqrt(D)).transpose(0, 2, 1)
        ).astype(bf)
        kt = np.ascontiguousarray(k4[sl].transpose(0, 2, 1)).astype(bf)
        vb = v4[sl].reshape(HPC, NKT, KT, D).astype(bf)
        vones = np.concatenate(
            [vb, np.ones((HPC, NKT, KT, 1), dtype=bf)], axis=-1
        )  # [HPC, NKT, KT, DV]
        v_lay = np.ascontiguousarray(vones.transpose(0, 2, 1, 3))  # [HPC, KT, NKT, DV]
        in_maps.append({
            "qt": qt,
            "kt": kt,
            "v": v_lay,
            "tri": tri,
        })
    return in_maps


def _assemble(per_core_results):
    out = np.stack([r["o"] for r in per_core_results])  # [8, HPC, S, D]
    return np.ascontiguousarray(
        out.reshape(B, H, S, D)
    ).astype(np.float32)


def kernel(query, key, value, causal_mask):
    import os
    os.environ["BASS_NEVER_TRACE"] = "1"  # axon NTFF hook unavailable here
    from concourse.bass_utils import run_bass_kernel_spmd

    global last_results
    causal = bool(np.asarray(causal_mask).item())
    if causal not in _cache:
        _cache[causal] = _build(causal)
    nc = _cache[causal]

    in_maps = _make_in_maps(query, key, value)
    res = run_bass_kernel_spmd(nc, in_maps, core_ids=list(range(NCORES)))
    last_results = res
    return _assemble(res.results)


# revision 24
# speedup vs baseline: 1.3599x; 1.3599x over previous
"""Causal flash attention for trn2: B=4,H=16,S=4096,D=64 fp32.

Sharding: 64 (b,h) heads -> 8 per NeuronCore, no cross-core comm.
Host prep (not counted in HW time): Q/K transposed to [d,s] bf16 (Q
pre-scaled by 1/sqrt(D)), V pre-laid-out [128, nkt, D+1] bf16 with an
appended ones-column so the PV matmul also produces the softmax
normalizer.

On-chip per head, per 512-query block j (PSUM: 3x double-buffered
2-bank score sets + 2 single-bank output accumulators = 8 banks):
  key tiles t<=4j+3 processed in groups of 2:
    ST[k=128, 1024] = two QK matmuls (contraction d=64, bf16, N=512)
    exp in ONE instruction per group, alternating engines:
      ACT: exp activation (table)         -> pt bf16
      DVE: Schraudolph bit-trick exp (x*A+B -> int16, bits = bf16)
    diagonal tiles: pt[:, dg*128:+128] *= upper-tri mask (DVE)
    PV: per 128-query sub s: O[q=128, s*65..+65] += PT_s^T @ [V_t|1]
        (single PSUM bank holds all 4 sub-accumulators; one start=True
        clears has_written once, later first-touches overwrite)
  epilogue: strided reciprocal of the 4 normalizer cols, 4 scaled
  copies, one DMA of [128, 4, 64] -> out rows.
"""

import math
from contextlib import ExitStack

import numpy as np
import ml_dtypes

B, H, S, D = 4, 16, 4096, 64
NCORES = 8
HPC = (B * H) // NCORES  # heads per core
QB = 512                 # query block
KT = 128                 # key tile (PE partition dim)
NKT = S // KT            # 32 key tiles per head
GT = 2                   # key tiles per exp group
DV = D + 1               # value cols + normalizer ones-column

# Schraudolph exp constants for bf16 bit pattern (7-bit mantissa):
# bits = round(x * 128/ln2 + (127*128 - 366393/65536))
SCH_A = 128.0 / math.log(2.0)
SCH_B = 127 * 128 - 366393.0 / 65536.0

# exp engine schedule: cycle over groups; ~3:2 ACT:DVE
EXP_PATTERN = ("act", "dve", "act", "dve", "act")

_cache = {}


def _build(causal: bool, hpc: int = HPC, s_len: int = S):
    import concourse.tile as tile
    from concourse import bacc, mybir

    f32 = mybir.dt.float32
    f16 = mybir.dt.float16
    bf16 = mybir.dt.bfloat16
    i16 = mybir.dt.int16
    EXP = mybir.ActivationFunctionType.Exp
    MULT = mybir.AluOpType.mult
    ADD = mybir.AluOpType.add
    nkt_total = s_len // KT
    nqb = s_len // QB

    nc = bacc.Bacc("TRN2", target_bir_lowering=False)
    qt_d = nc.dram_tensor("qt", [hpc, 2 * D, s_len], bf16, kind="ExternalInput")
    kt_d = nc.dram_tensor("kt", [hpc, 2 * D, s_len], bf16, kind="ExternalInput")
    v_d = nc.dram_tensor("v", [hpc, KT, nkt_total, DV], bf16, kind="ExternalInput")
    tri_d = nc.dram_tensor("tri", [KT, 2 * KT], bf16, kind="ExternalInput")
    o_d = nc.dram_tensor("o", [hpc, s_len, D], f32, kind="ExternalOutput")

    with ExitStack() as ctx:
        tc = ctx.enter_context(tile.TileContext(nc))
        qk_pool = ctx.enter_context(tc.tile_pool(name="qk", bufs=2))
        v_pool = ctx.enter_context(tc.tile_pool(name="v", bufs=2))
        p_pool = ctx.enter_context(tc.tile_pool(name="p", bufs=8))
        st_pool = ctx.enter_context(tc.tile_pool(name="st", bufs=3, space="PSUM"))
        o_pool = ctx.enter_context(tc.tile_pool(name="oacc", bufs=2, space="PSUM"))
        out_pool = ctx.enter_context(tc.tile_pool(name="out", bufs=4))
        const_pool = ctx.enter_context(tc.tile_pool(name="const", bufs=1))

        tri_t = const_pool.tile([KT, 2 * KT], bf16)
        nc.sync.dma_start(out=tri_t, in_=tri_d[:])

        gcounter = 0
        carry = []        # PV-pending groups of the previous block
        carry_epi = None  # (h, j, o_ps) of the previous block
        for h in range(hpc):
            qt_t = qk_pool.tile([2 * D, s_len], bf16, tag="qt")
            kt_t = qk_pool.tile([2 * D, s_len], bf16, tag="kt")
            if h == 0:
                # split the cold-start loads so block j=0 (first 1024 cols)
                # can begin while the rest streams in
                c = 2 * QB
                nc.sync.dma_start(out=qt_t[:, :c], in_=qt_d[h, :, :c])
                nc.sync.dma_start(out=kt_t[:, :c], in_=kt_d[h, :, :c])
                nc.sync.dma_start(out=qt_t[:, c:], in_=qt_d[h, :, c:])
                nc.sync.dma_start(out=kt_t[:, c:], in_=kt_d[h, :, c:])
            else:
                nc.sync.dma_start(out=qt_t, in_=qt_d[h])
                nc.sync.dma_start(out=kt_t, in_=kt_d[h])
            v_t = v_pool.tile([KT, nkt_total, DV], bf16, tag="v")
            nc.sync.dma_start(out=v_t, in_=v_d[h])

            # Cross-block software pipeline (carried across heads too): a
            # block's trailing PV groups and its epilogue are emitted
            # interleaved between the next block's QK/exp groups, so the
            # in-order PE stream always has independent QK work between PV
            # ops that wait on exp results.
            for j in range(nqb):
                o_ps = o_pool.tile([KT, 4 * DV], f32, tag="oacc", name=f"o_{h}_{j}")
                nkt = 4 * (j + 1) if causal else nkt_total
                groups = [
                    list(range(g0, min(g0 + GT, nkt))) for g0 in range(0, nkt, GT)
                ]
                pts = []
                for ts in groups:
                    w = len(ts) * QB
                    # skip the leading fully-masked columns of the group's
                    # first tile (nothing reads them); interior stale spans
                    # of later tiles still get exp'd harmlessly
                    dg0 = ts[0] - 4 * j if causal else -1
                    e0 = max(dg0, 0) * KT
                    st = st_pool.tile([KT, GT * QB], f32, tag="st")
                    for i, t in enumerate(ts):
                        dg = t - 4 * j if causal else -1
                        # diagonal tiles: only queries >= dg*128 can see keys
                        # of tile t; skip the fully-masked cols (stale PSUM
                        # there is finite, exp'd harmlessly, never read by PV)
                        q0 = max(dg, 0) * KT
                        r0 = (i % 2) * D  # PE row-group half for concurrency
                        nc.tensor.matmul(
                            st[:, i * QB + q0:(i + 1) * QB],
                            kt_t[r0:r0 + D, t * KT:(t + 1) * KT],
                            qt_t[r0:r0 + D, j * QB + q0:(j + 1) * QB],
                            start=True, stop=(dg < 0),
                        )
                        if dg >= 0:
                            # add -60 to masked (future-key) entries of the
                            # diagonal 128x128 block: identity.T @ mask_neg
                            c0 = i * QB + dg * KT
                            nc.tensor.matmul(
                                st[:, c0:c0 + KT],
                                tri_t[:, :KT],
                                tri_t[:, KT:2 * KT],
                                start=False, stop=True,
                            )
                    pt = p_pool.tile([KT, GT * QB], bf16, tag="pt")
                    if EXP_PATTERN[gcounter % len(EXP_PATTERN)] == "dve":
                        nc.vector.tensor_scalar(
                            pt.bitcast(i16)[:, e0:w], st[:, e0:w], SCH_A, SCH_B,
                            MULT, ADD,
                        )
                    else:
                        nc.scalar.activation(pt[:, e0:w], st[:, e0:w], EXP)
                    gcounter += 1
                    pts.append((j, o_ps, ts, pt, v_t))
                    if carry:
                        _emit_pv(nc, causal, carry.pop(0))
                        if not carry and carry_epi is not None:
                            _emit_epilogue(nc, out_pool, o_d, carry_epi, f32)
                            carry_epi = None
                    elif len(pts) >= 5:
                        _emit_pv(nc, causal, pts.pop(0))
                # previous block fully drained by now (it has fewer
                # groups than this block); stash this block's backlog
                for grp in carry:
                    _emit_pv(nc, causal, grp)
                if carry_epi is not None:
                    _emit_epilogue(nc, out_pool, o_d, carry_epi, f32)
                carry = pts
                carry_epi = (h, j, o_ps)
        for grp in carry:
            _emit_pv(nc, causal, grp)
        if carry_epi is not None:
            _emit_epilogue(nc, out_pool, o_d, carry_epi, f32)
    nc.compile()
    return nc


def _emit_pv(nc, causal, group):
    j, o_ps, ts, pt, v_t = group
    for i, t in enumerate(ts):
        dg = t - 4 * j if causal else -1
        for s in range(4):
            if dg > s:
                continue
            nc.tensor.matmul(
                o_ps[:, s * DV:(s + 1) * DV],
                pt[:, i * QB + s * KT:i * QB + (s + 1) * KT],
                v_t[:, t, :],
                start=(t == 0 and s == 0),
                stop=(t == (4 * j + s if causal else NKT - 1)),
            )


def _emit_epilogue(nc, out_pool, o_d, epi, f32):
    h, j, o_ps = epi
    recip = out_pool.tile([KT, 4], f32, tag="recip", name=f"r_{h}_{j}")
    nc.vector.reciprocal(recip, o_ps[:, D::DV])
    out_t = out_pool.tile([KT, 4, D], f32, tag="out", name=f"t_{h}_{j}")
    for s in range(4):
        nc.vector.tensor_scalar_mul(
            out_t[:, s], o_ps[:, s * DV:s * DV + D], recip[:, s:s + 1]
        )
    # issued from the idle GPSIMD queue so stores never block input-prefetch
    # DMAs queued on SP
    nc.gpsimd.dma_start(
        out=o_d[h, j * QB:(j + 1) * QB, :].rearrange("(s p) d -> p s d", s=4),
        in_=out_t,
    )


last_results = None  # BassKernelResults of the most recent run (for test.py)


def _make_in_maps(query, key, value):
    bf = ml_dtypes.bfloat16
    q4 = np.asarray(query, dtype=np.float32).reshape(B * H, S, D)
    k4 = np.asarray(key, dtype=np.float32).reshape(B * H, S, D)
    v4 = np.asarray(value, dtype=np.float32).reshape(B * H, S, D)
    # [identity | strict-lower-tri * -60]: operands of the mask-add matmul
    # (identity.T @ mask adds -60 where query < key inside a diagonal block)
    tri = np.concatenate(
        [np.eye(KT, dtype=np.float32),
         np.tril(np.full((KT, KT), -60.0, dtype=np.float32), -1)], axis=1
    ).astype(bf)

    in_maps = []
    for c in range(NCORES):
        sl = slice(c * HPC, (c + 1) * HPC)
        qt1 = (q4[sl] / math.sqrt(D)).transpose(0, 2, 1)
        qt = np.ascontiguousarray(
            np.concatenate([qt1, qt1], axis=1)
        ).astype(bf)
        kt1 = k4[sl].transpose(0, 2, 1)
        kt = np.ascontiguousarray(
            np.concatenate([kt1, kt1], axis=1)
        ).astype(bf)
        vb = v4[sl].reshape(HPC, NKT, KT, D).astype(bf)
        vones = np.concatenate(
            [vb, np.ones((HPC, NKT, KT, 1), dtype=bf)], axis=-1
        )  # [HPC, NKT, KT, DV]
        v_lay = np.ascontiguousarray(vones.transpose(0, 2, 1, 3))  # [HPC, KT, NKT, DV]
        in_maps.append({
            "qt": qt,
            "kt": kt,
            "v": v_lay,
            "tri": tri,
        })
    return in_maps


def _assemble(per_core_results):
    out = np.stack([r["o"] for r in per_core_results])  # [8, HPC, S, D]
    return np.ascontiguousarray(
        out.reshape(B, H, S, D)
    ).astype(np.float32)


def kernel(query, key, value, causal_mask):
    import os
    os.environ["BASS_NEVER_TRACE"] = "1"  # axon NTFF hook unavailable here
    from concourse.bass_utils import run_bass_kernel_spmd

    global last_results
    causal = bool(np.asarray(causal_mask).item())
    if causal not in _cache:
        _cache[causal] = _build(causal)
    nc = _cache[causal]

    in_maps = _make_in_maps(query, key, value)
    res = run_bass_kernel_spmd(nc, in_maps, core_ids=list(range(NCORES)))
    last_results = res
    return _assemble(res.results)
